# revision 37
# baseline (speedup 1.0000x reference)
"""Trainium2 Bass kernel for nn_CentersDistance (retrieval_knn).

logits[k, n] = -||centers[k] - inputs[n]||^2
             = 2*(centers @ inputs.T)[k, n] - ||centers[k]||^2 - ||inputs[n]||^2

Strategy (8 NeuronCores, data-parallel over the N=8192 inputs):
  * host: transpose both operands so the contraction dim D lands on the SBUF
    partition axis, fold the factor 2 into the inputs, quantize both to
    fp8e4m3, and precompute the norm terms exactly in float64.
  * device (per core): a 1024x1024x1024 matmul in fp8 DoubleRow mode
    (2 contraction rows/cycle on the PE = 157 TF/s, 2x the bf16 rate).
    DoubleRow packs two contraction sub-rows per partition: operands are
    laid out [128, 2, free] per 256-deep d-super-tile (4 tiles cover
    D=1024), so the whole per-core product is 64 matmuls x 512 moving
    rows = 32768 PE cycles = 13.7 us of PE stream.
  * epilogue adds the exact norm terms (-csq per-partition scalar, -xsq
    broadcast row) with scalar_tensor_tensor, split across the DVE
    (even groups) and Pool/GpSimd (odd groups) engines so the ~0.7 us
    per-group PSUM-read cost never becomes the critical path; output is
    written fp16 (the norm terms dominate the logits, measured absmax
    error stays ~5e-3 of scale) and upconverted to fp32 on the host.
    fp16 stores also halve the output DMA traffic: all queues share the
    same 16 DMA engines (~368 GB/s per core total), so with fp8 loads
    (2 MB) + fp16 stores (2 MB) + norm tiles the total DMA time stays
    under the PE stream time.
  * raw Block/semaphore implementation (not Tile), same skeleton as the
    earlier bf16 version: PE warmup matmuls open the HAM clock gate
    while loads stream; pass 1 (m-tiles 0-3) runs d outermost to pace
    with the streaming loads across 8 PSUM banks; pass 2 (m-tiles 4-7)
    runs d innermost so groups retire early and their epilogue + store
    overlap the remaining matmuls.  Bank reuse in pass 2 waits on the
    corresponding epilogue (concurrent PE-write + DVE-read of a PSUM
    bank is fatal on P10).
  * loads stream on two HW-DGE queues (Sync: xt, Scalar: ct) with one
    semaphore per d-tile pair; the norm tiles ride the GpSimd queue.
    Stores go out one m-tile (two groups, [128, 1024] fp16 = 256 KB) at
    a time, even m on Sync, odd m on Scalar, final m-tile split so the
    two halves land on both queues.

Previous bf16 version measured 44.2 us NEFF exec; ~27.6 us of that was
the bf16 PE-stream floor, plus ~8.5 us fixed NRT pre/postamble
(51-semaphore reset chains per engine) that this version keeps paying.
"""

import threading
from contextlib import ExitStack

import numpy as np
import ml_dtypes

import concourse.mybir as mybir
from concourse import bacc
import concourse.bass_utils as _bu
from concourse.bass_utils import run_bass_kernel_spmd
from contextlib import contextmanager


@contextmanager
def _ldw_opt_enabled():
    """Compile with walrus redundant-LDWEIGHTS elimination.  The kernel
    is laid out so consecutive matmuls share stationary weights (h-tile
    pairs), which halves the PE queue's LDWEIGHTS issue traffic — the
    stream is issue-rate-bound, not PE-stream-bound.  Scoped: only this
    kernel's walrus invocation sees the flag."""
    orig = _bu.run_command

    def patched(argv, **kwargs):
        argv = [
            "--enable-ldw-opt=true" if a == "--enable-ldw-opt=false" else a
            for a in argv
        ]
        return orig(argv, **kwargs)

    _bu.run_command = patched
    try:
        yield
    finally:
        _bu.run_command = orig

N_CORES = 8
N, K, D = 8192, 1024, 1024
NSH = N // N_CORES  # per-core slab of inputs
P = 128             # SBUF partitions
NF = 512            # matmul moving free dim (one fp32 PSUM bank)

DR = 2              # DoubleRow: contraction sub-rows per partition
DT_SUPER = P * DR   # 256 contraction rows per d-super-tile
D_TILES = D // DT_SUPER  # 4 contraction super-tiles
M_TILES = K // P    # 8 center tiles
H_TILES = NSH // NF # 2 moving-dim tiles

G = M_TILES * H_TILES  # 16 output groups of [128, 512]
GP1 = 8                # groups 0-7 -> pass 1 (m-tiles 0-3), banks 0-7
N_WU = 9               # PE warm-up matmuls

_DT = mybir.dt.float8e4
_NP_DT = ml_dtypes.float8_e4m3
_OUT_DT = mybir.dt.float16

_cache = threading.local()


def _g_mh(g):
    return g // H_TILES, g % H_TILES


def _build_nc():
    nc = bacc.Bacc(
        "TRN2", target_bir_lowering=False, debug=False, num_devices=N_CORES
    )
    # host pre-interleaved DoubleRow layouts: [t, p, (m|h, i, free)] with
    # logical contraction index d = t*256 + i*128 + p.  The per-matmul
    # operand block [2, 128|512] is CONTIGUOUS within each partition so
    # the LDWEIGHTS/moving APs are simple 2-level patterns.
    ct = nc.dram_tensor("ct", [D_TILES, P, DR * K], _DT, kind="ExternalInput").ap()
    xt = nc.dram_tensor("xt", [D_TILES, P, DR * NSH], _DT, kind="ExternalInput").ap()
    ncsq = nc.dram_tensor(
        "ncsq", [P, M_TILES], mybir.dt.float32, kind="ExternalInput"
    ).ap()
    nxsq = nc.dram_tensor(
        "nxsq", [P, NSH], mybir.dt.float32, kind="ExternalInput"
    ).ap()
    out = nc.dram_tensor("out", [K, NSH], _OUT_DT, kind="ExternalOutput").ap()

    out_r = out.rearrange("(m p) n -> m p n", p=P)

    def _ct_op(ct_sb_d, m):
        """[128, 2, 128] contiguous stationary block for m-tile m."""
        return ct_sb_d[:, m * DR * P : (m + 1) * DR * P].rearrange(
            "p (i k) -> p i k", i=DR
        )

    def _xt_op(xt_sb_d, h):
        """[128, 2, 512] contiguous moving block for h-tile h."""
        return xt_sb_d[:, h * DR * NF : (h + 1) * DR * NF].rearrange(
            "p (i n) -> p i n", i=DR
        )

    with (
        nc.sbuf_tensor("wu_sb", [P, DR * NF], _DT) as wu_sb,
        nc.sbuf_tensor("ncsq_sb", [P, M_TILES], mybir.dt.float32) as ncsq_sb,
        nc.sbuf_tensor("nxsq_sb", [P, NSH], mybir.dt.float32) as nxsq_sb,
        nc.sbuf_tensor("ot_sb", [P, G * NF], _OUT_DT) as ot_sb,
        # double-buffered staging for the Act->GpSimd even-group pipeline
        nc.sbuf_tensor("tmp_sb", [P, 2 * NF], _OUT_DT) as tmp_sb,
        ExitStack() as stack,
        nc.semaphore("const_sem") as const_sem,
        nc.semaphore("mm_sem") as mm_sem,
        nc.semaphore("v_sem") as v_sem,   # odd-group epilogues (DVE, from PSUM)
        nc.semaphore("a_sem") as a_sem,   # even-group PSUM->SBUF copies (Act)
        nc.semaphore("g_sem") as g_sem,   # even-group epilogues (GpSimd, from SBUF)
        nc.semaphore("dma_out") as dma_out,
        nc.Block() as block,
    ):
        d_sems = [
            stack.enter_context(nc.semaphore(f"d_sem{i}")) for i in range(D_TILES)
        ]
        ct_sb = [
            stack.enter_context(nc.sbuf_tensor(f"ct_sb{d}", [P, DR * K], _DT))
            for d in range(D_TILES)
        ]
        xt_sb = [
            stack.enter_context(nc.sbuf_tensor(f"xt_sb{d}", [P, DR * NSH], _DT))
            for d in range(D_TILES)
        ]
        ps = [
            stack.enter_context(nc.psum_tensor(f"ps{b}", [P, NF], mybir.dt.float32))
            for b in range(8)
        ]

        def epi_wait(engine, g):
            """Wait until the epilogue for group g has drained its bank."""
            if g % 2 == 0:
                engine.wait_ge(a_sem, g // 2 + 1)
            else:
                engine.wait_ge(v_sem, g // 2 + 1)

        @block.sync
        def _(sync):
            for d in range(D_TILES):
                sync.dma_start(xt_sb[d][:], xt[d]).then_inc(d_sems[d], 16)
            # nxsq AFTER the PE-pacing loads; first consumer is the
            # first epilogue at ~pass-1 end
            sync.dma_start(nxsq_sb[:], nxsq).then_inc(const_sem, 16)
            # stores: one m-tile = groups 2m, 2m+1 = [128, 1024] fp16
            for m in (0, 2):
                sync.wait_ge(g_sem, m + 1)
                sync.wait_ge(v_sem, m + 1)
                sync.dma_start(
                    out_r[m], ot_sb[:, 2 * m * NF : (2 * m + 2) * NF]
                ).then_inc(dma_out, 16)
            sync.wait_ge(v_sem, 6)  # groups 8, 9
            sync.dma_start(
                out_r[4], ot_sb[:, 8 * NF : 10 * NF]
            ).then_inc(dma_out, 16)
            # m=7 h=0 (group 14, on DVE) as soon as its epilogue lands
            sync.wait_ge(v_sem, 11)
            sync.dma_start(
                out_r[7][:, 0:NF], ot_sb[:, 14 * NF : 15 * NF]
            ).then_inc(dma_out, 16)
            # second half of group 15 — the other final store rides the
            # Scalar queue so the two halves complete in parallel
            sync.wait_ge(v_sem, 13)
            sync.dma_start(
                out_r[7][:, NF + NF // 2 : 2 * NF],
                ot_sb[:, 15 * NF + NF // 2 : 16 * NF],
            ).then_inc(dma_out, 16)
            sync.wait_ge(dma_out, 10 * 16)

        @block.scalar
        def _(scalar):
            for d in range(D_TILES):
                scalar.dma_start(ct_sb[d][:], ct[d]).then_inc(d_sems[d], 16)
            # even-group epilogue stage 1: Act drains the PSUM bank into
            # fp16 staging (GpSimd cannot read PSUM on TRN2); the odd-m
            # stores are interleaved in expected-readiness order so they
            # are not program-order-blocked behind late copies
            def copy_j(j):
                g = 2 * j
                m, _ = _g_mh(g)
                scalar.wait_ge(mm_sem, g + 1)
                if j >= 2:
                    scalar.wait_ge(g_sem, j - 1)  # staging slot free
                # drain the bank adding the per-partition -csq on the way
                nc.scalar.activation(
                    tmp_sb[:, (j % 2) * NF : (j % 2 + 1) * NF],
                    ps[g % 8][:],
                    mybir.ActivationFunctionType.Identity,
                    bias=ncsq_sb[:, m : m + 1],
                    scale=1.0,
                ).then_inc(a_sem, 1)

            def store_m(m):
                scalar.wait_ge(g_sem, m + 1)
                scalar.wait_ge(v_sem, m + 1)
                scalar.dma_start(
                    out_r[m], ot_sb[:, 2 * m * NF : (2 * m + 2) * NF]
                ).then_inc(dma_out, 16)

            for j in range(4):
                copy_j(j)
            store_m(1)
            store_m(3)
            scalar.wait_ge(v_sem, 8)  # groups 10, 11
            scalar.dma_start(
                out_r[5], ot_sb[:, 10 * NF : 12 * NF]
            ).then_inc(dma_out, 16)
            scalar.wait_ge(v_sem, 10)  # groups 12, 13
            scalar.dma_start(
                out_r[6], ot_sb[:, 12 * NF : 14 * NF]
            ).then_inc(dma_out, 16)
            # m=7 h=1 first half (group 15, on DVE)
            scalar.wait_ge(v_sem, 12)
            scalar.dma_start(
                out_r[7][:, NF : NF + NF // 2],
                ot_sb[:, 15 * NF : 15 * NF + NF // 2],
            ).then_inc(dma_out, 16)

        @block.gpsimd
        def _(gpsimd):
            gpsimd.dma_start(ncsq_sb[:], ncsq).then_inc(const_sem, 16)
            # HAM pre-ramp: the core clock ramps on activity; the PE
            # cannot start until its block entry (~1.5us in), so burn
            # idle-engine cycles immediately to open the clock early.
            for i in range(3):
                nc.gpsimd.memset(ot_sb[:, i * 2048 : (i + 1) * 2048], 0)
            gpsimd.wait_ge(const_sem, 32)
            # even-group epilogue stage 2: add the -xsq row from staging
            # (the -csq term was already added by Act's bias)
            for j in range(4):
                g = 2 * j
                _, h = _g_mh(g)
                gpsimd.wait_ge(a_sem, j + 1)
                nc.gpsimd.tensor_tensor(
                    ot_sb[:, g * NF : (g + 1) * NF],
                    tmp_sb[:, (j % 2) * NF : (j % 2 + 1) * NF],
                    nxsq_sb[:, h * NF : (h + 1) * NF],
                    op=mybir.AluOpType.add,
                ).then_inc(g_sem, 1)

        @block.tensor
        def _(tensor):
            # warm-up: open the HAM clock gate while the loads stream.
            # wu_sb is deliberately uninitialized — only PE-busy time
            # matters; bank 7 is rewritten with start=True by group 7.
            # The weight slice ALTERNATES so redundant-LDWEIGHTS
            # elimination cannot dedupe the warmups: a quiet PE queue
            # ramps the HAM clock measurably later (full clock at
            # ~12.6us vs ~10.4us observed).
            for i in range(N_WU):
                nc.tensor.matmul(
                    ps[GP1 - 1][:],
                    _ct_op(wu_sb, i % 4),
                    _xt_op(wu_sb, 0),
                    start=True,
                    stop=True,
                    perf_mode=mybir.MatmulPerfMode.DoubleRow,
                )
            # pass 1: groups 0-7 accumulate in banks 0-7, d outermost so
            # matmuls pace with the streaming loads
            for d in range(D_TILES):
                tensor.wait_ge(d_sems[d], 32)
                for g in range(GP1):
                    m, h = _g_mh(g)
                    mm = nc.tensor.matmul(
                        ps[g][:],
                        _ct_op(ct_sb[d], m),
                        _xt_op(xt_sb[d], h),
                        start=(d == 0),
                        stop=(d == D_TILES - 1),
                        perf_mode=mybir.MatmulPerfMode.DoubleRow,
                    )
                    if d == D_TILES - 1:
                        mm.then_inc(mm_sem, 1)
            # pass 2: m-tiles 4-7 reuse banks 0-7 once the epilogues have
            # drained the pass-1 groups from those banks.  The two h-tiles
            # of one m are interleaved per d so consecutive matmuls share
            # the same weights (redundant-LDWEIGHTS elimination fodder).
            for m in range(4, M_TILES - 1):
                g0 = 2 * m  # even group (h=0); odd is g0+1
                epi_wait(tensor, g0 - 8)
                epi_wait(tensor, g0 - 7)
                for d in range(D_TILES):
                    for h in range(H_TILES):
                        mm = nc.tensor.matmul(
                            ps[(g0 + h) % 8][:],
                            _ct_op(ct_sb[d], m),
                            _xt_op(xt_sb[d], h),
                            start=(d == 0),
                            stop=(d == D_TILES - 1),
                            perf_mode=mybir.MatmulPerfMode.DoubleRow,
                        )
                        if d == D_TILES - 1:
                            mm.then_inc(mm_sem, 1)
            # last m-tile stays group-sequential so group 14 retires four
            # matmuls before group 15 and the two tail epilogues overlap
            for g in (14, 15):
                m, h = _g_mh(g)
                epi_wait(tensor, g - 8)
                for d in range(D_TILES):
                    mm = nc.tensor.matmul(
                        ps[g % 8][:],
                        _ct_op(ct_sb[d], m),
                        _xt_op(xt_sb[d], h),
                        start=(d == 0),
                        stop=(d == D_TILES - 1),
                        perf_mode=mybir.MatmulPerfMode.DoubleRow,
                    )
                mm.then_inc(mm_sem, 1)
            # tail dummies: the HAM drops the core clock a few us after
            # the PE idles, which would run the epilogue/store tail and
            # the start of the fixed NRT postamble at reduced clock.
            # Keep the PE streaming throwaway matmuls (bank 0 is drained
            # once group 8's epilogue is done) until roughly when the
            # final store completes.
            tensor.wait_ge(v_sem, 5)
            for i in range(8):
                nc.tensor.matmul(
                    ps[0][:],
                    _ct_op(wu_sb, i % 4),
                    _xt_op(wu_sb, 0),
                    start=True,
                    stop=True,
                    perf_mode=mybir.MatmulPerfMode.DoubleRow,
                )

        @block.vector
        def _(vector):
            # HAM pre-ramp (see gpsimd): tmp_sb is not read until the
            # first even-group epilogue, long after these finish
            for _ in range(3):
                nc.vector.tensor_scalar_add(tmp_sb[:], nxsq_sb[:], 0.0)
            vector.wait_ge(const_sem, 32)  # ncsq + nxsq present
            # odd groups plus the four tail groups 12-15: the DVE reads
            # PSUM directly (~0.75us/group), so the kernel tail is two
            # back-to-back DVE ops instead of the slower Act->Pool chain
            for g in (1, 3, 5, 7, 8, 9, 10, 11, 12, 13, 14):
                m, h = _g_mh(g)
                vector.wait_ge(mm_sem, g + 1)
                nc.vector.scalar_tensor_tensor(
                    ot_sb[:, g * NF : (g + 1) * NF],
                    ps[g % 8][:],
                    ncsq_sb[:, m : m + 1],
                    nxsq_sb[:, h * NF : (h + 1) * NF],
                    op0=mybir.AluOpType.add,
                    op1=mybir.AluOpType.add,
                ).then_inc(v_sem, 1)
            # the last group is split in half so its first store can go
            # out while the second half is still draining
            HNF = NF // 2
            vector.wait_ge(mm_sem, G)
            for half in range(2):
                lo = 15 * NF + half * HNF
                nc.vector.scalar_tensor_tensor(
                    ot_sb[:, lo : lo + HNF],
                    ps[7][:, half * HNF : (half + 1) * HNF],
                    ncsq_sb[:, 7:8],
                    nxsq_sb[:, NF + half * HNF : NF + (half + 1) * HNF],
                    op0=mybir.AluOpType.add,
                    op1=mybir.AluOpType.add,
                ).then_inc(v_sem, 1)

    nc.compile()
    return nc


def _get_nc():
    if not hasattr(_cache, "nc"):
        _cache.nc = _build_nc()
    return _cache.nc


def _to_double_row(a, blk):
    """[D, F] -> [D_TILES, P, DR*F] with d = t*256 + i*128 + p and the
    free axis grouped as (block, i, f%blk) so each per-matmul operand
    block [DR, blk] is contiguous within a partition."""
    f = a.shape[1]
    return np.ascontiguousarray(
        a.reshape(D_TILES, DR, P, f // blk, blk)
        .transpose(0, 2, 3, 1, 4)
        .reshape(D_TILES, P, DR * f)
    )


def _to_sw_weights(a):
    """[D, K] -> DoubleRowSwInterleave weights layout: per (t, p,
    m)-block the 256 values are (i=0, i=1) pairs interleaved per output
    column with the columns stored in reverse order:
    flat[2*(127-k) + i] = W[i, k]."""
    w = a.reshape(D_TILES, DR, P, M_TILES, P).transpose(0, 2, 3, 4, 1)
    w = w[:, :, :, ::-1, :]  # reverse k within each m-block
    return np.ascontiguousarray(w.reshape(D_TILES, P, DR * K))


def kernel(inputs, centers, _trace=False):
    inputs = np.asarray(inputs, dtype=np.float32)
    centers = np.asarray(centers, dtype=np.float32)

    csq = np.sum(centers.astype(np.float64) ** 2, axis=1)
    xsq = np.sum(inputs.astype(np.float64) ** 2, axis=1)

    ct = _to_double_row(np.ascontiguousarray(centers.T).astype(_NP_DT), P)
    xt2 = np.ascontiguousarray((2.0 * inputs).T.astype(_NP_DT))
    ncsq = np.ascontiguousarray((-csq).reshape(M_TILES, P).T.astype(np.float32))

    in_maps = []
    for i in range(N_CORES):
        sl = slice(i * NSH, (i + 1) * NSH)
        in_maps.append(
            {
                "ct": ct,
                "xt": _to_double_row(np.ascontiguousarray(xt2[:, sl]), NF),
                "ncsq": ncsq,
                "nxsq": np.ascontiguousarray(
                    np.broadcast_to(-xsq[sl], (P, NSH))
                ).astype(np.float32),
            }
        )

    nc = _get_nc()
    try:
        with _ldw_opt_enabled():
            res = run_bass_kernel_spmd(
                nc, in_maps, core_ids=list(range(N_CORES)), trace=_trace
            )
    except ModuleNotFoundError:
        # NTFF trace glue is absent in some images; rerun without tracing
        with _ldw_opt_enabled():
            res = run_bass_kernel_spmd(
                nc, in_maps, core_ids=list(range(N_CORES)), trace=False
            )
    if _trace:
        kernel.last_results = res
    return np.concatenate(
        [r["out"].astype(np.float32) for r in res.results], axis=1
    )


# revision 38
# speedup vs baseline: 1.0154x; 1.0154x over previous
"""Trainium2 Bass kernel for nn_CentersDistance (retrieval_knn).

logits[k, n] = -||centers[k] - inputs[n]||^2
             = 2*(centers @ inputs.T)[k, n] - ||centers[k]||^2 - ||inputs[n]||^2

Strategy (8 NeuronCores, data-parallel over the N=8192 inputs):
  * host: transpose both operands so the contraction dim D lands on the SBUF
    partition axis, fold the factor 2 into the inputs, quantize both to
    fp8e4m3, and precompute the norm terms exactly in float64.
  * device (per core): a 1024x1024x1024 matmul in fp8 DoubleRow mode
    (2 contraction rows/cycle on the PE = 157 TF/s, 2x the bf16 rate).
    DoubleRow packs two contraction sub-rows per partition: operands are
    laid out [128, 2, free] per 256-deep d-super-tile (4 tiles cover
    D=1024), so the whole per-core product is 64 matmuls x 512 moving
    rows = 32768 PE cycles = 13.7 us of PE stream.
  * epilogue adds the exact norm terms (-csq per-partition scalar, -xsq
    broadcast row) with scalar_tensor_tensor, split across the DVE
    (even groups) and Pool/GpSimd (odd groups) engines so the ~0.7 us
    per-group PSUM-read cost never becomes the critical path; output is
    written fp16 (the norm terms dominate the logits, measured absmax
    error stays ~5e-3 of scale) and upconverted to fp32 on the host.
    fp16 stores also halve the output DMA traffic: all queues share the
    same 16 DMA engines (~368 GB/s per core total), so with fp8 loads
    (2 MB) + fp16 stores (2 MB) + norm tiles the total DMA time stays
    under the PE stream time.
  * raw Block/semaphore implementation (not Tile), same skeleton as the
    earlier bf16 version: PE warmup matmuls open the HAM clock gate
    while loads stream; pass 1 (m-tiles 0-3) runs d outermost to pace
    with the streaming loads across 8 PSUM banks; pass 2 (m-tiles 4-7)
    runs d innermost so groups retire early and their epilogue + store
    overlap the remaining matmuls.  Bank reuse in pass 2 waits on the
    corresponding epilogue (concurrent PE-write + DVE-read of a PSUM
    bank is fatal on P10).
  * loads stream on two HW-DGE queues (Sync: xt, Scalar: ct) with one
    semaphore per d-tile pair; the norm tiles ride the GpSimd queue.
    Stores go out one m-tile (two groups, [128, 1024] fp16 = 256 KB) at
    a time, even m on Sync, odd m on Scalar, final m-tile split so the
    two halves land on both queues.

Previous bf16 version measured 44.2 us NEFF exec; ~27.6 us of that was
the bf16 PE-stream floor, plus ~8.5 us fixed NRT pre/postamble
(51-semaphore reset chains per engine) that this version keeps paying.
"""

import threading
from contextlib import ExitStack

import numpy as np
import ml_dtypes

import concourse.mybir as mybir
from concourse import bacc
import concourse.bass_utils as _bu
from concourse.bass_utils import run_bass_kernel_spmd
from contextlib import contextmanager


@contextmanager
def _ldw_opt_enabled():
    """Compile with walrus redundant-LDWEIGHTS elimination.  The kernel
    is laid out so consecutive matmuls share stationary weights (h-tile
    pairs), which halves the PE queue's LDWEIGHTS issue traffic — the
    stream is issue-rate-bound, not PE-stream-bound.  Scoped: only this
    kernel's walrus invocation sees the flag."""
    orig = _bu.run_command

    def patched(argv, **kwargs):
        argv = [
            "--enable-ldw-opt=true" if a == "--enable-ldw-opt=false" else a
            for a in argv
        ]
        return orig(argv, **kwargs)

    _bu.run_command = patched
    try:
        yield
    finally:
        _bu.run_command = orig

N_CORES = 8
N, K, D = 8192, 1024, 1024
NSH = N // N_CORES  # per-core slab of inputs
P = 128             # SBUF partitions
NF = 512            # matmul moving free dim (one fp32 PSUM bank)

DR = 2              # DoubleRow: contraction sub-rows per partition
DT_SUPER = P * DR   # 256 contraction rows per d-super-tile
D_TILES = D // DT_SUPER  # 4 contraction super-tiles
M_TILES = K // P    # 8 center tiles
H_TILES = NSH // NF # 2 moving-dim tiles

G = M_TILES * H_TILES  # 16 output groups of [128, 512]
GP1 = 8                # groups 0-7 -> pass 1 (m-tiles 0-3), banks 0-7
N_WU = 9               # PE warm-up matmuls

_DT = mybir.dt.float8e4
_NP_DT = ml_dtypes.float8_e4m3
_OUT_DT = mybir.dt.float16

_cache = threading.local()


def _g_mh(g):
    return g // H_TILES, g % H_TILES


def _build_nc():
    nc = bacc.Bacc(
        "TRN2", target_bir_lowering=False, debug=False, num_devices=N_CORES
    )
    # host pre-interleaved DoubleRow layouts: [t, p, (m|h, i, free)] with
    # logical contraction index d = t*256 + i*128 + p.  The per-matmul
    # operand block [2, 128|512] is CONTIGUOUS within each partition so
    # the LDWEIGHTS/moving APs are simple 2-level patterns.
    ct = nc.dram_tensor("ct", [D_TILES, P, DR * K], _DT, kind="ExternalInput").ap()
    xt = nc.dram_tensor("xt", [D_TILES, P, DR * NSH], _DT, kind="ExternalInput").ap()
    ncsq = nc.dram_tensor(
        "ncsq", [P, M_TILES], mybir.dt.float32, kind="ExternalInput"
    ).ap()
    nxsq = nc.dram_tensor(
        "nxsq", [P, NSH], mybir.dt.float32, kind="ExternalInput"
    ).ap()
    out = nc.dram_tensor("out", [K, NSH], _OUT_DT, kind="ExternalOutput").ap()

    out_r = out.rearrange("(m p) n -> m p n", p=P)

    def _ct_op(ct_sb_d, m):
        """[128, 2, 128] contiguous stationary block for m-tile m."""
        return ct_sb_d[:, m * DR * P : (m + 1) * DR * P].rearrange(
            "p (i k) -> p i k", i=DR
        )

    def _xt_op(xt_sb_d, h):
        """[128, 2, 512] contiguous moving block for h-tile h."""
        return xt_sb_d[:, h * DR * NF : (h + 1) * DR * NF].rearrange(
            "p (i n) -> p i n", i=DR
        )

    with (
        nc.sbuf_tensor("wu_sb", [P, DR * NF], _DT) as wu_sb,
        nc.sbuf_tensor("ncsq_sb", [P, M_TILES], mybir.dt.float32) as ncsq_sb,
        nc.sbuf_tensor("nxsq_sb", [P, NSH], mybir.dt.float32) as nxsq_sb,
        nc.sbuf_tensor("ot_sb", [P, G * NF], _OUT_DT) as ot_sb,
        # double-buffered staging for the Act->GpSimd even-group pipeline
        nc.sbuf_tensor("tmp_sb", [P, 2 * NF], _OUT_DT) as tmp_sb,
        ExitStack() as stack,
        nc.semaphore("const_sem") as const_sem,
        nc.semaphore("mm_sem") as mm_sem,
        nc.semaphore("v_sem") as v_sem,   # odd-group epilogues (DVE, from PSUM)
        nc.semaphore("a_sem") as a_sem,   # even-group PSUM->SBUF copies (Act)
        nc.semaphore("g_sem") as g_sem,   # even-group epilogues (GpSimd, from SBUF)
        nc.semaphore("dma_out") as dma_out,
        nc.Block() as block,
    ):
        d_sems = [
            stack.enter_context(nc.semaphore(f"d_sem{i}")) for i in range(D_TILES)
        ]
        ct_sb = [
            stack.enter_context(nc.sbuf_tensor(f"ct_sb{d}", [P, DR * K], _DT))
            for d in range(D_TILES)
        ]
        xt_sb = [
            stack.enter_context(nc.sbuf_tensor(f"xt_sb{d}", [P, DR * NSH], _DT))
            for d in range(D_TILES)
        ]
        ps = [
            stack.enter_context(nc.psum_tensor(f"ps{b}", [P, NF], mybir.dt.float32))
            for b in range(8)
        ]

        def epi_wait(engine, g):
            """Wait until the epilogue for group g has drained its bank."""
            if g % 2 == 0:
                engine.wait_ge(a_sem, g // 2 + 1)
            else:
                engine.wait_ge(v_sem, g // 2 + 1)

        @block.sync
        def _(sync):
            for d in range(D_TILES):
                sync.dma_start(xt_sb[d][:], xt[d]).then_inc(d_sems[d], 16)
            # nxsq AFTER the PE-pacing loads; first consumer is the
            # first epilogue at ~pass-1 end
            sync.dma_start(nxsq_sb[:], nxsq).then_inc(const_sem, 16)
            # stores: one m-tile = groups 2m, 2m+1 = [128, 1024] fp16
            for m in (0, 2):
                sync.wait_ge(g_sem, m + 1)
                sync.wait_ge(v_sem, m + 1)
                sync.dma_start(
                    out_r[m], ot_sb[:, 2 * m * NF : (2 * m + 2) * NF]
                ).then_inc(dma_out, 16)
            sync.wait_ge(v_sem, 6)  # groups 8, 9
            sync.dma_start(
                out_r[4], ot_sb[:, 8 * NF : 10 * NF]
            ).then_inc(dma_out, 16)
            # m=7 h=0 (group 14, on DVE) as soon as its epilogue lands
            sync.wait_ge(v_sem, 11)
            sync.dma_start(
                out_r[7][:, 0:NF], ot_sb[:, 14 * NF : 15 * NF]
            ).then_inc(dma_out, 16)
            # second half of group 15 — the other final store rides the
            # Scalar queue so the two halves complete in parallel
            sync.wait_ge(v_sem, 13)
            sync.dma_start(
                out_r[7][:, NF + NF // 2 : 2 * NF],
                ot_sb[:, 15 * NF + NF // 2 : 16 * NF],
            ).then_inc(dma_out, 16)
            sync.wait_ge(dma_out, 10 * 16)

        @block.scalar
        def _(scalar):
            for d in range(D_TILES):
                scalar.dma_start(ct_sb[d][:], ct[d]).then_inc(d_sems[d], 16)
            # even-group epilogue stage 1: Act drains the PSUM bank into
            # fp16 staging (GpSimd cannot read PSUM on TRN2); the odd-m
            # stores are interleaved in expected-readiness order so they
            # are not program-order-blocked behind late copies
            def copy_j(j):
                g = 2 * j
                m, _ = _g_mh(g)
                scalar.wait_ge(mm_sem, g + 1)
                if j >= 2:
                    scalar.wait_ge(g_sem, j - 1)  # staging slot free
                # drain the bank adding the per-partition -csq on the way
                nc.scalar.activation(
                    tmp_sb[:, (j % 2) * NF : (j % 2 + 1) * NF],
                    ps[g % 8][:],
                    mybir.ActivationFunctionType.Identity,
                    bias=ncsq_sb[:, m : m + 1],
                    scale=1.0,
                ).then_inc(a_sem, 1)

            def store_m(m):
                scalar.wait_ge(g_sem, m + 1)
                scalar.wait_ge(v_sem, m + 1)
                scalar.dma_start(
                    out_r[m], ot_sb[:, 2 * m * NF : (2 * m + 2) * NF]
                ).then_inc(dma_out, 16)

            for j in range(4):
                copy_j(j)
            store_m(1)
            store_m(3)
            scalar.wait_ge(v_sem, 8)  # groups 10, 11
            scalar.dma_start(
                out_r[5], ot_sb[:, 10 * NF : 12 * NF]
            ).then_inc(dma_out, 16)
            scalar.wait_ge(v_sem, 10)  # groups 12, 13
            scalar.dma_start(
                out_r[6], ot_sb[:, 12 * NF : 14 * NF]
            ).then_inc(dma_out, 16)
            # m=7 h=1 first half (group 15, on DVE)
            scalar.wait_ge(v_sem, 12)
            scalar.dma_start(
                out_r[7][:, NF : NF + NF // 2],
                ot_sb[:, 15 * NF : 15 * NF + NF // 2],
            ).then_inc(dma_out, 16)

        @block.gpsimd
        def _(gpsimd):
            gpsimd.dma_start(ncsq_sb[:], ncsq).then_inc(const_sem, 16)
            gpsimd.wait_ge(const_sem, 32)
            # even-group epilogue stage 2: add the -xsq row from staging
            # (the -csq term was already added by Act's bias)
            for j in range(4):
                g = 2 * j
                _, h = _g_mh(g)
                gpsimd.wait_ge(a_sem, j + 1)
                nc.gpsimd.tensor_tensor(
                    ot_sb[:, g * NF : (g + 1) * NF],
                    tmp_sb[:, (j % 2) * NF : (j % 2 + 1) * NF],
                    nxsq_sb[:, h * NF : (h + 1) * NF],
                    op=mybir.AluOpType.add,
                ).then_inc(g_sem, 1)

        @block.tensor
        def _(tensor):
            # warm-up: open the HAM clock gate while the loads stream.
            # wu_sb is deliberately uninitialized — only PE-busy time
            # matters; bank 7 is rewritten with start=True by group 7.
            # The weight slice ALTERNATES so redundant-LDWEIGHTS
            # elimination cannot dedupe the warmups: a quiet PE queue
            # ramps the HAM clock measurably later (full clock at
            # ~12.6us vs ~10.4us observed).
            for i in range(N_WU):
                nc.tensor.matmul(
                    ps[GP1 - 1][:],
                    _ct_op(wu_sb, i % 4),
                    _xt_op(wu_sb, 0),
                    start=True,
                    stop=True,
                    perf_mode=mybir.MatmulPerfMode.DoubleRow,
                )
            # pass 1: groups 0-7 accumulate in banks 0-7, d outermost so
            # matmuls pace with the streaming loads
            for d in range(D_TILES):
                tensor.wait_ge(d_sems[d], 32)
                for g in range(GP1):
                    m, h = _g_mh(g)
                    mm = nc.tensor.matmul(
                        ps[g][:],
                        _ct_op(ct_sb[d], m),
                        _xt_op(xt_sb[d], h),
                        start=(d == 0),
                        stop=(d == D_TILES - 1),
                        perf_mode=mybir.MatmulPerfMode.DoubleRow,
                    )
                    if d == D_TILES - 1:
                        mm.then_inc(mm_sem, 1)
            # pass 2: m-tiles 4-7 reuse banks 0-7 once the epilogues have
            # drained the pass-1 groups from those banks.  The two h-tiles
            # of one m are interleaved per d so consecutive matmuls share
            # the same weights (redundant-LDWEIGHTS elimination fodder).
            for m in range(4, M_TILES - 1):
                g0 = 2 * m  # even group (h=0); odd is g0+1
                epi_wait(tensor, g0 - 8)
                epi_wait(tensor, g0 - 7)
                for d in range(D_TILES):
                    for h in range(H_TILES):
                        mm = nc.tensor.matmul(
                            ps[(g0 + h) % 8][:],
                            _ct_op(ct_sb[d], m),
                            _xt_op(xt_sb[d], h),
                            start=(d == 0),
                            stop=(d == D_TILES - 1),
                            perf_mode=mybir.MatmulPerfMode.DoubleRow,
                        )
                        if d == D_TILES - 1:
                            mm.then_inc(mm_sem, 1)
            # last m-tile stays group-sequential so group 14 retires four
            # matmuls before group 15 and the two tail epilogues overlap
            for g in (14, 15):
                m, h = _g_mh(g)
                epi_wait(tensor, g - 8)
                for d in range(D_TILES):
                    mm = nc.tensor.matmul(
                        ps[g % 8][:],
                        _ct_op(ct_sb[d], m),
                        _xt_op(xt_sb[d], h),
                        start=(d == 0),
                        stop=(d == D_TILES - 1),
                        perf_mode=mybir.MatmulPerfMode.DoubleRow,
                    )
                mm.then_inc(mm_sem, 1)
            # tail dummies: the HAM drops the core clock a few us after
            # the PE idles, which would run the epilogue/store tail and
            # the start of the fixed NRT postamble at reduced clock.
            # Keep the PE streaming throwaway matmuls (bank 0 is drained
            # once group 8's epilogue is done) until roughly when the
            # final store completes.
            tensor.wait_ge(v_sem, 5)
            for i in range(8):
                nc.tensor.matmul(
                    ps[0][:],
                    _ct_op(wu_sb, i % 4),
                    _xt_op(wu_sb, 0),
                    start=True,
                    stop=True,
                    perf_mode=mybir.MatmulPerfMode.DoubleRow,
                )

        @block.vector
        def _(vector):
            vector.wait_ge(const_sem, 32)  # ncsq + nxsq present
            # odd groups plus the four tail groups 12-15: the DVE reads
            # PSUM directly (~0.75us/group), so the kernel tail is two
            # back-to-back DVE ops instead of the slower Act->Pool chain
            for g in (1, 3, 5, 7, 8, 9, 10, 11, 12, 13, 14):
                m, h = _g_mh(g)
                vector.wait_ge(mm_sem, g + 1)
                nc.vector.scalar_tensor_tensor(
                    ot_sb[:, g * NF : (g + 1) * NF],
                    ps[g % 8][:],
                    ncsq_sb[:, m : m + 1],
                    nxsq_sb[:, h * NF : (h + 1) * NF],
                    op0=mybir.AluOpType.add,
                    op1=mybir.AluOpType.add,
                ).then_inc(v_sem, 1)
            # the last group is split in half so its first store can go
            # out while the second half is still draining
            HNF = NF // 2
            vector.wait_ge(mm_sem, G)
            for half in range(2):
                lo = 15 * NF + half * HNF
                nc.vector.scalar_tensor_tensor(
                    ot_sb[:, lo : lo + HNF],
                    ps[7][:, half * HNF : (half + 1) * HNF],
                    ncsq_sb[:, 7:8],
                    nxsq_sb[:, NF + half * HNF : NF + (half + 1) * HNF],
                    op0=mybir.AluOpType.add,
                    op1=mybir.AluOpType.add,
                ).then_inc(v_sem, 1)

    nc.compile()
    return nc


def _get_nc():
    if not hasattr(_cache, "nc"):
        _cache.nc = _build_nc()
    return _cache.nc


def _to_double_row(a, blk):
    """[D, F] -> [D_TILES, P, DR*F] with d = t*256 + i*128 + p and the
    free axis grouped as (block, i, f%blk) so each per-matmul operand
    block [DR, blk] is contiguous within a partition."""
    f = a.shape[1]
    return np.ascontiguousarray(
        a.reshape(D_TILES, DR, P, f // blk, blk)
        .transpose(0, 2, 3, 1, 4)
        .reshape(D_TILES, P, DR * f)
    )


def _to_sw_weights(a):
    """[D, K] -> DoubleRowSwInterleave weights layout: per (t, p,
    m)-block the 256 values are (i=0, i=1) pairs interleaved per output
    column with the columns stored in reverse order:
    flat[2*(127-k) + i] = W[i, k]."""
    w = a.reshape(D_TILES, DR, P, M_TILES, P).transpose(0, 2, 3, 4, 1)
    w = w[:, :, :, ::-1, :]  # reverse k within each m-block
    return np.ascontiguousarray(w.reshape(D_TILES, P, DR * K))


def kernel(inputs, centers, _trace=False):
    inputs = np.asarray(inputs, dtype=np.float32)
    centers = np.asarray(centers, dtype=np.float32)

    csq = np.sum(centers.astype(np.float64) ** 2, axis=1)
    xsq = np.sum(inputs.astype(np.float64) ** 2, axis=1)

    ct = _to_double_row(np.ascontiguousarray(centers.T).astype(_NP_DT), P)
    xt2 = np.ascontiguousarray((2.0 * inputs).T.astype(_NP_DT))
    ncsq = np.ascontiguousarray((-csq).reshape(M_TILES, P).T.astype(np.float32))

    in_maps = []
    for i in range(N_CORES):
        sl = slice(i * NSH, (i + 1) * NSH)
        in_maps.append(
            {
                "ct": ct,
                "xt": _to_double_row(np.ascontiguousarray(xt2[:, sl]), NF),
                "ncsq": ncsq,
                "nxsq": np.ascontiguousarray(
                    np.broadcast_to(-xsq[sl], (P, NSH))
                ).astype(np.float32),
            }
        )

    nc = _get_nc()
    try:
        with _ldw_opt_enabled():
            res = run_bass_kernel_spmd(
                nc, in_maps, core_ids=list(range(N_CORES)), trace=_trace
            )
    except ModuleNotFoundError:
        # NTFF trace glue is absent in some images; rerun without tracing
        with _ldw_opt_enabled():
            res = run_bass_kernel_spmd(
                nc, in_maps, core_ids=list(range(N_CORES)), trace=False
            )
    if _trace:
        kernel.last_results = res
    return np.concatenate(
        [r["out"].astype(np.float32) for r in res.results], axis=1
    )
